# revision 56
# baseline (speedup 1.0000x reference)
"""Trainium2 Bass kernel for nn_GraphPatchEmbed (patch-embed conv + GCN layer).

Math: the whole module is linear in x.
  feats = patches(x) @ Wc.T            (2x2/stride-2 conv == per-patch matmul, K=12)
  xw    = feats @ gcn_w                -> xw = patches @ (Wc.T @ gcn_w) = P @ Wcomb
  out   = D^-1/2 (A+I') D^-1/2 xw + b  (graph aggregation; edges only touch batch 0)
Aggregation (node axis) and matmul (channel axis) commute, so the stencil is applied
on the host to the 12-row patch tensor, the bias folds in as a 13th all-ones row,
and the device kernel is one memory-bound matmul per core:
  [32768, 13] @ [13, 192]   (8-way row-sharded over B*N)

Device design (v5, emb-major / W-stationary):
  - The bottleneck is PSUM evacuation: only DVE and ACT can read PSUM
    (~1 elem/cycle/partition each; GpSimd has no PSUM port, DMA has no
    PSUM route), and each copy instruction pays a flat PSUM/SBUF access
    penalty. So the goal is full 512-wide PSUM banks and long contiguous
    copies.
  - Stationary = W columns ([13,128] for emb 0:128, [13,64] for emb
    128:192), moving = q [13, 512 nodes] -> each matmul fills one whole
    2KB PSUM bank with a single accumulation group.
  - The PE streams ~0.83 ns/col per row-band (MID p-state, never
    ramps), but matmuls in DIFFERENT row quadrants stream fully
    concurrently (measured: 2 bands 0.42 ns/col, 3 bands 0.28). W and
    q live at partition bases {0, 32, 64}, chunk c in band c%3, so
    consecutive matmuls always overlap and the PE (~18us) stays well
    under the copy bound (~28us).
  - Phase 2 (emb 128:192, M=64) packs two node-chunks per bank at
    output partition bases 0 and 64 so copies always span 128
    partitions.
  - PSUM is one flat [128, 8*512] tile; matmuls and 2-bank copies
    rotate through it, relying on Tile's subtile dependency tracking.
  - Copies alternate DVE (CAST) / ACT (COPY) with a slight ACT bias
    (ACT is 0.83 ns/elem vs DVE 1.04).
  - W rides in the first 192 columns of the q tensor so one DMA primes
    both W and the first node chunks; the q load ramps in chunks on the
    sync queue ahead of all output DMAs.
  - Output fp8e3 with a x4 pre-scale folded into W (host decodes).
"""

import numpy as np

from concourse import bacc, mybir, tile
import concourse.bass as bass
from concourse.bass_utils import run_bass_kernel_spmd

B, CIN, HIMG, WIMG = 4, 3, 512, 512
HG, WG = 256, 256          # grid after 2x2/stride-2 patching
N = HG * WG                # 65536 nodes per image
BN = B * N                 # 262144 total rows
EMB = 192
K = 13                     # 12 patch dims + 1 bias row
NCORES = 8
ROWS = BN // NCORES        # 32768 rows per core
FP8_SCALE = 4.0            # folded into W before the e3m4 downcast

CHUNK = 512                # node-cols per matmul == one full psum bank
NCHUNK = ROWS // CHUNK     # 64 chunks per core
WCOLS = EMB                # W header columns of each strip
NBAND = 3
BASES = (0, 32, 64)        # PE row-band per chunk: chunk c -> BASES[c % 3]
LCH = [(NCHUNK + NBAND - 1 - s) // NBAND for s in range(NBAND)]  # chunks/strip
SCOLS = WCOLS + max(LCH) * CHUNK   # 11456 columns per strip
QCOLS = NBAND * SCOLS              # dram q: [strip0 | strip1 | strip2]

NBANK = 8                  # psum banks; ring of 4 groups x 2 banks
GROUP = 2 * CHUNK          # elems per copy (2 banks)
NR1 = NCHUNK // 2          # 32 phase-1 rounds (2 chunks/round)
NR2 = NCHUNK // 4          # 16 phase-2 rounds (4 chunks/round)
NROUND = NR1 + NR2         # 48 rounds; round r -> banks (2*(r%4), +1)
OCOLS = NROUND * GROUP     # 49152 output cols
SGRP = 2                   # rounds per staging tile / output DMA

# input ramp, per strip: SYNC_RAMP levels on the sync queue (earliest),
# GP_RAMP levels on the gpsimd SWDGE queue (runs in parallel). Each
# dispatch costs 0.65-1.15us of serial queue time.
SYNC_RAMP = [WCOLS + 512, 1024]
GP_RAMP = [2048, 4096, 3584]
assert sum(SYNC_RAMP) + sum(GP_RAMP) == SCOLS

# ARR[c] = (strip, local) slot of node-chunk c, ordered by predicted
# DMA-arrival time so consumption order == arrival order (the Tile
# scheduler orders matmuls by modeled arrival; any mismatch stalls the
# in-order copy pipeline). Sync dispatches land ~0.8us apart from ~8.3;
# gpsimd dispatches run on a parallel queue from ~9.8, ~1.15us apart.
def _arrival_order():
    events = []  # (time, [(strip, local), ...])
    t = 8.3
    for li, csz in enumerate(SYNC_RAMP):
        base_l = sum(SYNC_RAMP[:li]) - WCOLS
        locals_ = range(max(0, base_l // CHUNK),
                        (base_l + csz) // CHUNK if li else 1)
        for s in range(NBAND):
            events.append((t, [(s, l) for l in locals_ if l < LCH[s]]))
            t += 0.8
    t = 9.8
    off = sum(SYNC_RAMP) - WCOLS
    for csz in GP_RAMP:
        lo, hi = off // CHUNK, (off + csz) // CHUNK
        for s in range(NBAND):
            events.append((t, [(s, l) for l in range(lo, hi) if l < LCH[s]]))
            t += 1.15
        off += csz
    events.sort(key=lambda e: e[0])
    return [sl for _, sls in events for sl in sls]

ARR = _arrival_order()
assert len(ARR) == NCHUNK and len(set(ARR)) == NCHUNK

# phase 2 rereads chunks (all resident by then), so its consumption
# order is free: use strict band rotation so consecutive matmuls
# overlap in different PE row-bands (ARR's same-band runs would make
# phase 2 PE-bound at ~1.7us/round vs the 1.12us copy cadence).
_INV = {sl: c for c, sl in enumerate(ARR)}
P2C = [_INV[(j % NBAND, j // NBAND)] for j in range(NBAND * max(LCH))
       if j // NBAND < LCH[j % NBAND]]
assert len(P2C) == NCHUNK

# copy-engine assignment per round: 0 -> DVE, 1 -> ACT. ACT is slightly
# faster per element, so it takes 25 of 48.
ENG_OF = [(1 if r % 2 else 0) for r in range(NROUND)]
ENG_OF[24] = 1

# per-engine staging: each engine's rounds get consecutive o8 columns
# (engine-major), so an output DMA waits on ONE engine's copies only --
# a late copy on one engine can no longer head-of-line-block the other
# engine's output stream on the sync queue.
RND_OF = [[r for r in range(NROUND) if ENG_OF[r] == e] for e in (0, 1)]
ROUND_COL = [0] * NROUND
for _e in (0, 1):
    for _i, _r in enumerate(RND_OF[_e]):
        ROUND_COL[_r] = (_e * len(RND_OF[0]) + _i) * GROUP

_NC_CACHE = {}


def _build_nc(out_bufs=8, eng_of=None):
    eng_of = list(eng_of) if eng_of is not None else list(ENG_OF)
    key = (out_bufs, tuple(eng_of))
    if key in _NC_CACHE:
        return _NC_CACHE[key]
    nc = bacc.Bacc(
        "TRN2",
        target_bir_lowering=False,
        debug=False,
        enable_asserts=False,
        num_devices=NCORES,
        enable_partition_id=False,
    )
    f16 = mybir.dt.float16
    f32 = mybir.dt.float32
    f8 = mybir.dt.float8e3
    q = nc.dram_tensor("q", [K, NBAND * SCOLS], f16, kind="ExternalInput").ap()
    o8 = nc.dram_tensor("o8", [128, OCOLS], f8, kind="ExternalOutput").ap()

    with tile.TileContext(nc) as tc:
        with (
            tc.tile_pool(name="qp", bufs=1) as qpool,
            tc.tile_pool(name="ps", bufs=1, space=bass.MemorySpace.PSUM) as pspool,
            tc.tile_pool(name="ot", bufs=out_bufs) as opool,
        ):
            qw = qpool.tile([BASES[-1] + K, SCOLS], f16)
            # strip s lives at partition base BASES[s]; dispatch size-major
            # (all strips' level k before level k+1)
            offs = [0] * NBAND
            for ramp, eng in ((SYNC_RAMP, nc.sync), (GP_RAMP, nc.gpsimd)):
                for csz in ramp:
                    for s, base in enumerate(BASES):
                        off = offs[s]
                        eng.dma_start(
                            out=qw[base:base + K, off:off + csz],
                            in_=q[:, s * SCOLS + off:s * SCOLS + off + csz])
                        offs[s] += csz

            ps = pspool.tile([128, NBANK * CHUNK], f32)

            def mov(c):
                """moving AP for node-chunk c, slot ARR[c] = (strip, local)."""
                s, l = ARR[c]
                lo = WCOLS + l * CHUNK
                return qw[BASES[s]:BASES[s] + K, lo:lo + CHUNK]

            def w1(c):
                b = BASES[ARR[c][0]]
                return qw[b:b + K, 0:128]

            def w2(c):
                b = BASES[ARR[c][0]]
                return qw[b:b + K, 128:EMB]

            # per-engine staging tiles of 2 rounds (leftover -> 1-round
            # tile); a tile's DMA waits on one engine's copies only
            etile = [None, None]
            ecnt = [0, 0]
            ecap = [0, 0]
            edone = [0, 0]
            for r in range(NROUND):
                poff = (r % 4) * GROUP
                if r < NR1:
                    for kk in range(2):
                        c = 2 * r + kk
                        nc.tensor.matmul(
                            ps[:, poff + kk * CHUNK:poff + (kk + 1) * CHUNK],
                            w1(c), mov(c), start=True, stop=True,
                        )
                else:
                    for kk in range(2):
                        i0 = 4 * (r - NR1) + 2 * kk
                        ca, cb = P2C[i0], P2C[i0 + 1]
                        dst = ps[:, poff + kk * CHUNK:poff + (kk + 1) * CHUNK]
                        nc.tensor.matmul(
                            dst[0:64, :], w2(ca), mov(ca),
                            start=True, stop=True,
                        )
                        nc.tensor.matmul(
                            dst[64:128, :], w2(cb), mov(cb),
                            start=True, stop=True,
                        )
                e = eng_of[r]
                nleft = len(RND_OF[e]) - edone[e]
                if etile[e] is None:
                    ecap[e] = min(2, nleft)
                    etile[e] = opool.tile([128, ecap[e] * GROUP], f8,
                                          name=f"ot{e}")
                    ecnt[e] = 0
                src = ps[:, poff:poff + GROUP]
                dst = etile[e][:, ecnt[e] * GROUP:(ecnt[e] + 1) * GROUP]
                if e:
                    nc.scalar.copy(dst, src)
                else:
                    nc.vector.tensor_copy(dst, src)
                ecnt[e] += 1
                edone[e] += 1
                if ecnt[e] == ecap[e]:
                    col = ROUND_COL[r] - (ecnt[e] - 1) * GROUP
                    nc.sync.dma_start(
                        out=o8[:, col:col + ecnt[e] * GROUP], in_=etile[e][:])
                    etile[e] = None
    nc.compile()
    _NC_CACHE[key] = nc
    return nc


def _host_prep(x, conv_w, gcn_w, gcn_b):
    x = np.asarray(x, dtype=np.float32)
    conv_w = np.asarray(conv_w, dtype=np.float32)
    gcn_w = np.asarray(gcn_w, dtype=np.float32)
    gcn_b = np.asarray(gcn_b, dtype=np.float32)

    # patches P[b, k, n]: k = (cin, ki, kj), n = r*WG + c
    P = np.ascontiguousarray(
        x.reshape(B, CIN, HG, 2, WG, 2).transpose(0, 1, 3, 5, 2, 4)
    ).reshape(B, 12, N)

    # degrees with self-loops; grid edges exist only for batch 0
    nbr = np.full((HG, WG), 4.0, np.float32)
    nbr[0, :] -= 1; nbr[-1, :] -= 1; nbr[:, 0] -= 1; nbr[:, -1] -= 1
    deg = nbr + 1.0
    deg[HG - 2, WG - 2] += 1.0          # the module's trailing extra edge
    dr = (1.0 / np.sqrt(deg)).ravel()    # dinv per node

    # batch-0 aggregation applied to the patch rows (commutes with the matmul)
    z = (dr[None, :] * P[0]).reshape(12, HG, WG)
    s = z.copy()                          # self-loop term
    s[:, 1:, :] += z[:, :-1, :]
    s[:, :-1, :] += z[:, 1:, :]
    s[:, :, 1:] += z[:, :, :-1]
    s[:, :, :-1] += z[:, :, 1:]
    s[:, HG - 2, WG - 2] += z[:, HG - 1, WG - 1]
    Q0 = dr[None, :] * s.reshape(12, N)

    Q = np.empty((K, BN), np.float32)
    Q[:12, :N] = Q0
    Q[:12, N:] = P[1:].transpose(1, 0, 2).reshape(12, 3 * N)
    Q[12, :] = 1.0                        # bias row

    Wcomb = (conv_w.reshape(EMB, 12).astype(np.float64).T
             @ gcn_w.astype(np.float64)).astype(np.float32)
    Wfull = np.concatenate([Wcomb, gcn_b[None, :]], axis=0)  # (13, 192)
    return Q, Wfull


def _decode_core(o8core, inv):
    """[128, OCOLS] fp8 -> [ROWS, EMB] fp32 (engine-major round layout)."""
    o = o8core.astype(np.float32) * inv
    out = np.empty((ROWS, EMB), np.float32)
    for r in range(NROUND):
        seg = o[:, ROUND_COL[r]:ROUND_COL[r] + GROUP].reshape(128, 2, CHUNK)
        if r < NR1:
            # phase 1: partition p = emb p, bank kk = chunk 2r+kk
            for kk in range(2):
                c = 2 * r + kk
                out[c * CHUNK:(c + 1) * CHUNK, 0:128] = seg[:, kk].T
        else:
            # phase 2: bank kk holds chunks P2C[i0] (p 0:64), P2C[i0+1]
            for kk in range(2):
                i0 = 4 * (r - NR1) + 2 * kk
                ca, cb = P2C[i0], P2C[i0 + 1]
                out[ca * CHUNK:(ca + 1) * CHUNK, 128:EMB] = seg[0:64, kk].T
                out[cb * CHUNK:(cb + 1) * CHUNK, 128:EMB] = seg[64:128, kk].T
    return out


def kernel(x, conv_w, gcn_w, gcn_b, _trace=False, _nc_kwargs=None):
    Q, Wfull = _host_prep(x, conv_w, gcn_w, gcn_b)
    nc = _build_nc(**(_nc_kwargs or {}))
    W16 = (Wfull * FP8_SCALE).astype(np.float16)
    Q16 = Q.astype(np.float16)
    in_maps = []
    for c in range(NCORES):
        qc = Q16[:, c * ROWS:(c + 1) * ROWS].reshape(K, NCHUNK, CHUNK)
        qd = np.zeros((K, NBAND, SCOLS), np.float16)
        qd[:, :, :WCOLS] = W16[:, None, :]
        for ch in range(NCHUNK):
            s, l = ARR[ch]
            qd[:, s, WCOLS + l * CHUNK:WCOLS + (l + 1) * CHUNK] = qc[:, ch]
        in_maps.append({"q": qd.reshape(K, NBAND * SCOLS)})
    res = run_bass_kernel_spmd(nc, in_maps, list(range(NCORES)), trace=_trace)
    inv = np.float32(1.0 / FP8_SCALE)
    out = np.empty((NCORES, ROWS, EMB), np.float32)
    for c in range(NCORES):
        out[c] = _decode_core(res.results[c]["o8"], inv)
    out = out.reshape(B, N, EMB)
    if _trace:
        return out, res
    return out


# revision 58
# speedup vs baseline: 1.0256x; 1.0256x over previous
"""Trainium2 Bass kernel for nn_GraphPatchEmbed (patch-embed conv + GCN layer).

Math: the whole module is linear in x.
  feats = patches(x) @ Wc.T            (2x2/stride-2 conv == per-patch matmul, K=12)
  xw    = feats @ gcn_w                -> xw = patches @ (Wc.T @ gcn_w) = P @ Wcomb
  out   = D^-1/2 (A+I') D^-1/2 xw + b  (graph aggregation; edges only touch batch 0)
Aggregation (node axis) and matmul (channel axis) commute, so the stencil is applied
on the host to the 12-row patch tensor, the bias folds in as a 13th all-ones row,
and the device kernel is one memory-bound matmul per core:
  [32768, 13] @ [13, 192]   (8-way row-sharded over B*N)

Device design (v5, emb-major / W-stationary):
  - The bottleneck is PSUM evacuation: only DVE and ACT can read PSUM
    (~1 elem/cycle/partition each; GpSimd has no PSUM port, DMA has no
    PSUM route), and each copy instruction pays a flat PSUM/SBUF access
    penalty. So the goal is full 512-wide PSUM banks and long contiguous
    copies.
  - Stationary = W columns ([13,128] for emb 0:128, [13,64] for emb
    128:192), moving = q [13, 512 nodes] -> each matmul fills one whole
    2KB PSUM bank with a single accumulation group.
  - The PE streams ~0.83 ns/col per row-band (MID p-state, never
    ramps), but matmuls in DIFFERENT row quadrants stream fully
    concurrently (measured: 2 bands 0.42 ns/col, 3 bands 0.28). W and
    q live at partition bases {0, 32, 64}, chunk c in band c%3, so
    consecutive matmuls always overlap and the PE (~18us) stays well
    under the copy bound (~28us).
  - Phase 2 (emb 128:192, M=64) packs two node-chunks per bank at
    output partition bases 0 and 64 so copies always span 128
    partitions.
  - PSUM is one flat [128, 8*512] tile; matmuls and 2-bank copies
    rotate through it, relying on Tile's subtile dependency tracking.
  - Copies alternate DVE (CAST) / ACT (COPY) with a slight ACT bias
    (ACT is 0.83 ns/elem vs DVE 1.04).
  - W rides in the first 192 columns of the q tensor so one DMA primes
    both W and the first node chunks; the q load ramps in chunks on the
    sync queue ahead of all output DMAs.
  - Output fp8e3 with a x4 pre-scale folded into W (host decodes).
"""

import numpy as np

from concourse import bacc, mybir, tile
import concourse.bass as bass
from concourse.bass_utils import run_bass_kernel_spmd

B, CIN, HIMG, WIMG = 4, 3, 512, 512
HG, WG = 256, 256          # grid after 2x2/stride-2 patching
N = HG * WG                # 65536 nodes per image
BN = B * N                 # 262144 total rows
EMB = 192
K = 13                     # 12 patch dims + 1 bias row
NCORES = 8
ROWS = BN // NCORES        # 32768 rows per core
FP8_SCALE = 4.0            # folded into W before the e3m4 downcast

CHUNK = 512                # node-cols per matmul == one full psum bank
NCHUNK = ROWS // CHUNK     # 64 chunks per core
WCOLS = EMB                # W header columns of each strip
NBAND = 3
BASES = (0, 32, 64)        # PE row-band per chunk: chunk c -> BASES[c % 3]
LCH = [(NCHUNK + NBAND - 1 - s) // NBAND for s in range(NBAND)]  # chunks/strip
SCOLS = WCOLS + max(LCH) * CHUNK   # 11456 columns per strip
QCOLS = NBAND * SCOLS              # dram q: [strip0 | strip1 | strip2]

NBANK = 8                  # psum banks; ring of 4 groups x 2 banks
GROUP = 2 * CHUNK          # elems per copy (2 banks)
NR1 = NCHUNK // 2          # 32 phase-1 rounds (2 chunks/round)
NR2 = NCHUNK // 4          # 16 phase-2 rounds (4 chunks/round)
NROUND = NR1 + NR2         # 48 rounds; round r -> banks (2*(r%4), +1)
OCOLS = NROUND * GROUP     # 49152 output cols
SGRP = 2                   # rounds per staging tile / output DMA

# input ramp, per strip: SYNC_RAMP levels on the sync queue (earliest),
# GP_RAMP levels on the gpsimd SWDGE queue (runs in parallel). Each
# dispatch costs 0.65-1.15us of serial queue time.
SYNC_RAMP = [WCOLS + 512, 1024]
GP_RAMP = [2048, 4096, 3584]
assert sum(SYNC_RAMP) + sum(GP_RAMP) == SCOLS

# ARR[c] = (strip, local) slot of node-chunk c, ordered by predicted
# DMA-arrival time so consumption order == arrival order (the Tile
# scheduler orders matmuls by modeled arrival; any mismatch stalls the
# in-order copy pipeline). Sync dispatches land ~0.8us apart from ~8.3;
# gpsimd dispatches run on a parallel queue from ~9.8, ~1.15us apart.
def _arrival_order():
    events = []  # (time, [(strip, local), ...])
    t = 8.3
    for li, csz in enumerate(SYNC_RAMP):
        base_l = sum(SYNC_RAMP[:li]) - WCOLS
        locals_ = range(max(0, base_l // CHUNK),
                        (base_l + csz) // CHUNK if li else 1)
        for s in range(NBAND):
            events.append((t, [(s, l) for l in locals_ if l < LCH[s]]))
            t += 0.8
    t = 9.8
    off = sum(SYNC_RAMP) - WCOLS
    for csz in GP_RAMP:
        lo, hi = off // CHUNK, (off + csz) // CHUNK
        for s in range(NBAND):
            events.append((t, [(s, l) for l in range(lo, hi) if l < LCH[s]]))
            t += 1.15
        off += csz
    events.sort(key=lambda e: e[0])
    return [sl for _, sls in events for sl in sls]

ARR = _arrival_order()
assert len(ARR) == NCHUNK and len(set(ARR)) == NCHUNK

# phase 2 rereads chunks (all resident by then), so its consumption
# order is free: use strict band rotation so consecutive matmuls
# overlap in different PE row-bands (ARR's same-band runs would make
# phase 2 PE-bound at ~1.7us/round vs the 1.12us copy cadence).
_INV = {sl: c for c, sl in enumerate(ARR)}
P2C = [_INV[(j % NBAND, j // NBAND)] for j in range(NBAND * max(LCH))
       if j // NBAND < LCH[j % NBAND]]
assert len(P2C) == NCHUNK

# copy-engine assignment per round: 0 -> DVE, 1 -> ACT. ACT is slightly
# faster per element, so it takes 25 of 48.
ENG_OF = [(1 if r % 2 else 0) for r in range(NROUND)]
ENG_OF[24] = 1

# per-engine staging: each engine's copy items get consecutive o8
# columns (engine-major), so an output DMA waits on ONE engine's copies
# only -- a late copy on one engine can no longer head-of-line-block
# the other engine's output stream on the sync queue. Rounds 0, 1 and
# the last round are split into 1-bank items on BOTH engines so the
# pipeline starts and drains faster; other rounds are one 2-bank item
# on ENG_OF[r]. Items pack greedily into staging tiles of <= 4 banks.
ITEMS = [[], []]
for _r in range(NROUND):
    if _r < 2 or _r == NROUND - 1:
        ITEMS[0].append((_r, 0, 1))
        ITEMS[1].append((_r, 1, 1))
    else:
        ITEMS[ENG_OF[_r]].append((_r, 0, 2))
ITEM_COL = {}
_col = 0
for _e in (0, 1):
    for _r, _b, _nb in ITEMS[_e]:
        ITEM_COL[(_r, _b)] = _col
        _col += _nb * CHUNK
assert _col == OCOLS
TILES = [[], []]
for _e in (0, 1):
    cur, nb = [], 0
    for _it in ITEMS[_e]:
        if nb + _it[2] > 4:
            TILES[_e].append(cur)
            cur, nb = [], 0
        cur.append(_it)
        nb += _it[2]
    if cur:
        TILES[_e].append(cur)
LAST_OF_TILE = {t[-1][:2]: t for _e in (0, 1) for t in TILES[_e]}

_NC_CACHE = {}


def _build_nc(out_bufs=8, eng_of=None):
    eng_of = list(eng_of) if eng_of is not None else list(ENG_OF)
    key = (out_bufs, tuple(eng_of))
    if key in _NC_CACHE:
        return _NC_CACHE[key]
    nc = bacc.Bacc(
        "TRN2",
        target_bir_lowering=False,
        debug=False,
        enable_asserts=False,
        num_devices=NCORES,
        enable_partition_id=False,
    )
    f16 = mybir.dt.float16
    f32 = mybir.dt.float32
    f8 = mybir.dt.float8e3
    q = nc.dram_tensor("q", [K, NBAND * SCOLS], f16, kind="ExternalInput").ap()
    o8 = nc.dram_tensor("o8", [128, OCOLS], f8, kind="ExternalOutput").ap()

    with tile.TileContext(nc) as tc:
        with (
            tc.tile_pool(name="qp", bufs=1) as qpool,
            tc.tile_pool(name="ps", bufs=1, space=bass.MemorySpace.PSUM) as pspool,
            tc.tile_pool(name="ot", bufs=out_bufs) as opool,
        ):
            qw = qpool.tile([BASES[-1] + K, SCOLS], f16)
            # strip s lives at partition base BASES[s]; dispatch size-major
            # (all strips' level k before level k+1)
            offs = [0] * NBAND
            for ramp, eng in ((SYNC_RAMP, nc.sync), (GP_RAMP, nc.gpsimd)):
                for csz in ramp:
                    for s, base in enumerate(BASES):
                        off = offs[s]
                        eng.dma_start(
                            out=qw[base:base + K, off:off + csz],
                            in_=q[:, s * SCOLS + off:s * SCOLS + off + csz])
                        offs[s] += csz

            ps = pspool.tile([128, NBANK * CHUNK], f32)

            def mov(c):
                """moving AP for node-chunk c, slot ARR[c] = (strip, local)."""
                s, l = ARR[c]
                lo = WCOLS + l * CHUNK
                return qw[BASES[s]:BASES[s] + K, lo:lo + CHUNK]

            def w1(c):
                b = BASES[ARR[c][0]]
                return qw[b:b + K, 0:128]

            def w2(c):
                b = BASES[ARR[c][0]]
                return qw[b:b + K, 128:EMB]

            # per-engine staging tiles (<= 4 banks of items); a tile's
            # DMA waits on one engine's copies only
            etile = [None, None]
            eoff = [0, 0]
            for r in range(NROUND):
                poff = (r % 4) * GROUP
                if r < NR1:
                    for kk in range(2):
                        c = 2 * r + kk
                        nc.tensor.matmul(
                            ps[:, poff + kk * CHUNK:poff + (kk + 1) * CHUNK],
                            w1(c), mov(c), start=True, stop=True,
                        )
                else:
                    for kk in range(2):
                        i0 = 4 * (r - NR1) + 2 * kk
                        ca, cb = P2C[i0], P2C[i0 + 1]
                        dst = ps[:, poff + kk * CHUNK:poff + (kk + 1) * CHUNK]
                        nc.tensor.matmul(
                            dst[0:64, :], w2(ca), mov(ca),
                            start=True, stop=True,
                        )
                        nc.tensor.matmul(
                            dst[64:128, :], w2(cb), mov(cb),
                            start=True, stop=True,
                        )
                if r < 2 or r == NROUND - 1:
                    ritems = [(0, (r, 0, 1)), (1, (r, 1, 1))]
                else:
                    ritems = [(eng_of[r], (r, 0, 2))]
                for e, it in ritems:
                    _, b, nb = it
                    if etile[e] is None:
                        cap = sum(x[2] for x in
                                  next(t for t in TILES[e] if t[0] == it))
                        etile[e] = opool.tile([128, cap * CHUNK], f8,
                                              name=f"ot{e}")
                        eoff[e] = 0
                    src = ps[:, poff + b * CHUNK:poff + (b + nb) * CHUNK]
                    dst = etile[e][:, eoff[e]:eoff[e] + nb * CHUNK]
                    if e:
                        nc.scalar.copy(dst, src)
                    else:
                        nc.vector.tensor_copy(dst, src)
                    eoff[e] += nb * CHUNK
                    tile_items = LAST_OF_TILE.get((r, b))
                    if tile_items is not None and \
                            sum(x[2] for x in tile_items) * CHUNK == eoff[e]:
                        col = ITEM_COL[tile_items[0][:2]]
                        nc.sync.dma_start(
                            out=o8[:, col:col + eoff[e]], in_=etile[e][:])
                        etile[e] = None
    nc.compile()
    _NC_CACHE[key] = nc
    return nc


def _host_prep(x, conv_w, gcn_w, gcn_b):
    x = np.asarray(x, dtype=np.float32)
    conv_w = np.asarray(conv_w, dtype=np.float32)
    gcn_w = np.asarray(gcn_w, dtype=np.float32)
    gcn_b = np.asarray(gcn_b, dtype=np.float32)

    # patches P[b, k, n]: k = (cin, ki, kj), n = r*WG + c
    P = np.ascontiguousarray(
        x.reshape(B, CIN, HG, 2, WG, 2).transpose(0, 1, 3, 5, 2, 4)
    ).reshape(B, 12, N)

    # degrees with self-loops; grid edges exist only for batch 0
    nbr = np.full((HG, WG), 4.0, np.float32)
    nbr[0, :] -= 1; nbr[-1, :] -= 1; nbr[:, 0] -= 1; nbr[:, -1] -= 1
    deg = nbr + 1.0
    deg[HG - 2, WG - 2] += 1.0          # the module's trailing extra edge
    dr = (1.0 / np.sqrt(deg)).ravel()    # dinv per node

    # batch-0 aggregation applied to the patch rows (commutes with the matmul)
    z = (dr[None, :] * P[0]).reshape(12, HG, WG)
    s = z.copy()                          # self-loop term
    s[:, 1:, :] += z[:, :-1, :]
    s[:, :-1, :] += z[:, 1:, :]
    s[:, :, 1:] += z[:, :, :-1]
    s[:, :, :-1] += z[:, :, 1:]
    s[:, HG - 2, WG - 2] += z[:, HG - 1, WG - 1]
    Q0 = dr[None, :] * s.reshape(12, N)

    Q = np.empty((K, BN), np.float32)
    Q[:12, :N] = Q0
    Q[:12, N:] = P[1:].transpose(1, 0, 2).reshape(12, 3 * N)
    Q[12, :] = 1.0                        # bias row

    Wcomb = (conv_w.reshape(EMB, 12).astype(np.float64).T
             @ gcn_w.astype(np.float64)).astype(np.float32)
    Wfull = np.concatenate([Wcomb, gcn_b[None, :]], axis=0)  # (13, 192)
    return Q, Wfull


def _decode_core(o8core, inv):
    """[128, OCOLS] fp8 -> [ROWS, EMB] fp32 (item-major column layout)."""
    o = o8core.astype(np.float32) * inv
    out = np.empty((ROWS, EMB), np.float32)
    for e in (0, 1):
        for r, b, nb in ITEMS[e]:
            col = ITEM_COL[(r, b)]
            seg = o[:, col:col + nb * CHUNK].reshape(128, nb, CHUNK)
            for j in range(nb):
                kk = b + j
                if r < NR1:
                    c = 2 * r + kk
                    out[c * CHUNK:(c + 1) * CHUNK, 0:128] = seg[:, j].T
                else:
                    i0 = 4 * (r - NR1) + 2 * kk
                    ca, cb = P2C[i0], P2C[i0 + 1]
                    out[ca * CHUNK:(ca + 1) * CHUNK, 128:EMB] = \
                        seg[0:64, j].T
                    out[cb * CHUNK:(cb + 1) * CHUNK, 128:EMB] = \
                        seg[64:128, j].T
    return out


def kernel(x, conv_w, gcn_w, gcn_b, _trace=False, _nc_kwargs=None):
    Q, Wfull = _host_prep(x, conv_w, gcn_w, gcn_b)
    nc = _build_nc(**(_nc_kwargs or {}))
    W16 = (Wfull * FP8_SCALE).astype(np.float16)
    Q16 = Q.astype(np.float16)
    in_maps = []
    for c in range(NCORES):
        qc = Q16[:, c * ROWS:(c + 1) * ROWS].reshape(K, NCHUNK, CHUNK)
        qd = np.zeros((K, NBAND, SCOLS), np.float16)
        qd[:, :, :WCOLS] = W16[:, None, :]
        for ch in range(NCHUNK):
            s, l = ARR[ch]
            qd[:, s, WCOLS + l * CHUNK:WCOLS + (l + 1) * CHUNK] = qc[:, ch]
        in_maps.append({"q": qd.reshape(K, NBAND * SCOLS)})
    res = run_bass_kernel_spmd(nc, in_maps, list(range(NCORES)), trace=_trace)
    inv = np.float32(1.0 / FP8_SCALE)
    out = np.empty((NCORES, ROWS, EMB), np.float32)
    for c in range(NCORES):
        out[c] = _decode_core(res.results[c]["o8"], inv)
    out = out.reshape(B, N, EMB)
    if _trace:
        return out, res
    return out


# revision 74
# speedup vs baseline: 1.0449x; 1.0188x over previous
"""Trainium2 Bass kernel for nn_GraphPatchEmbed (patch-embed conv + GCN layer).

Math: the whole module is linear in x.
  feats = patches(x) @ Wc.T            (2x2/stride-2 conv == per-patch matmul, K=12)
  xw    = feats @ gcn_w                -> xw = patches @ (Wc.T @ gcn_w) = P @ Wcomb
  out   = D^-1/2 (A+I') D^-1/2 xw + b  (graph aggregation; edges only touch batch 0)
Aggregation (node axis) and matmul (channel axis) commute, so the stencil is applied
on the host to the 12-row patch tensor, the bias folds in as a 13th all-ones row,
and the device kernel is one memory-bound matmul per core:
  [32768, 13] @ [13, 192]   (8-way row-sharded over B*N)

Device design (v18, emb-major / W-stationary):
  - The bottleneck is PSUM evacuation: only DVE and ACT can read PSUM
    (~1 elem/cycle/partition each; GpSimd has no PSUM port, DMA has no
    PSUM route), and each copy instruction pays a flat PSUM/SBUF access
    penalty. So the goal is full 512-wide PSUM banks and long
    contiguous copies: steady state is ~1.75 Gelem/s/partition combined
    (~28us for the 48 rounds).
  - Stationary = W columns ([13,128] for emb 0:128, [13,64] for emb
    128:192), moving = q [13, 512 nodes] -> each matmul fills one whole
    2KB PSUM bank with a single accumulation group.
  - The PE streams ~0.83 ns/col per row-band (MID p-state, never
    ramps), but matmuls in DIFFERENT row quadrants stream fully
    concurrently (measured: 2 bands 0.42 ns/col, 3 bands 0.28). W and
    q are striped across partition bases {0, 32, 64}; phase 2 rereads
    chunks in strict band rotation so its 4 matmuls/round overlap.
  - Phase 2 (emb 128:192, M=64) packs two node-chunks per bank at
    output partition bases 0 and 64 so copies always span 128
    partitions (two accumulation groups per bank is fine on disjoint
    partition quadrants).
  - PSUM is one flat [128, 8*512] tile run as a ring of 4 two-bank
    groups (round r -> group r%4, engine r%2 -- strict parity, with
    one full parity flip at the phase-1/2 boundary, which measured
    ~0.7us better than a mid-phase flip or no flip), relying on Tile's
    subtile dependency tracking.
  - The Tile scheduler orders matmuls by MODELED data arrival, and the
    copies drain in round order, so node-chunk c is stored in the c-th
    slot to arrive (ARR): sync-queue dispatches land ~0.8us apart,
    gpsimd dispatches in parallel ~1.15us apart.
  - Per-engine staging: each engine's copies land in its own staging
    tiles and o8 column range, so an output DMA waits on one engine
    only (no cross-engine head-of-line blocking on the sync queue).
    Rounds 0, 1 and 47 split into 1-bank copies on both engines for a
    faster start/drain.
  - W rides in the first 192 columns of each strip so the first DMA
    primes both W and round 0.
  - Output fp8e3 with a x4 pre-scale folded into W (host decodes);
    measured rel err 1.334e-2 (all from the fp8 output quantization).
"""

import numpy as np

from concourse import bacc, mybir, tile
import concourse.bass as bass
from concourse.bass_utils import run_bass_kernel_spmd

B, CIN, HIMG, WIMG = 4, 3, 512, 512
HG, WG = 256, 256          # grid after 2x2/stride-2 patching
N = HG * WG                # 65536 nodes per image
BN = B * N                 # 262144 total rows
EMB = 192
K = 13                     # 12 patch dims + 1 bias row
NCORES = 8
ROWS = BN // NCORES        # 32768 rows per core
FP8_SCALE = 4.0            # folded into W before the e3m4 downcast

CHUNK = 512                # node-cols per matmul == one full psum bank
NCHUNK = ROWS // CHUNK     # 64 chunks per core
WCOLS = EMB                # W header columns of each strip
NBAND = 3
BASES = (0, 32, 64)        # PE row-band per chunk: chunk c -> BASES[c % 3]
LCH = [(NCHUNK + NBAND - 1 - s) // NBAND for s in range(NBAND)]  # chunks/strip
SCOLS = WCOLS + max(LCH) * CHUNK   # 11456 columns per strip
QCOLS = NBAND * SCOLS              # dram q: [strip0 | strip1 | strip2]

NBANK = 8                  # psum banks; ring of 4 groups x 2 banks
GROUP = 2 * CHUNK          # elems per copy (2 banks)
NR1 = NCHUNK // 2          # 32 phase-1 rounds (2 chunks/round)
NR2 = NCHUNK // 4          # 16 phase-2 rounds (4 chunks/round)
NROUND = NR1 + NR2         # 48 rounds; round r -> banks (2*(r%4), +1)
OCOLS = NROUND * GROUP     # 49152 output cols
SGRP = 2                   # rounds per staging tile / output DMA

# input ramp, per strip: SYNC_RAMP levels on the sync queue (earliest),
# GP_RAMP levels on the gpsimd SWDGE queue (runs in parallel). Each
# dispatch costs 0.65-1.15us of serial queue time.
SYNC_RAMP = [WCOLS + 512, 1024]
GP_RAMP = [1024, 2048, 4096, 2560]
assert sum(SYNC_RAMP) + sum(GP_RAMP) == SCOLS

# ARR[c] = (strip, local) slot of node-chunk c, ordered by predicted
# DMA-arrival time so consumption order == arrival order (the Tile
# scheduler orders matmuls by modeled arrival; any mismatch stalls the
# in-order copy pipeline). Sync dispatches land ~0.8us apart from ~8.3;
# gpsimd dispatches run on a parallel queue from ~9.8, ~1.15us apart.
def _arrival_order():
    events = []  # (time, [(strip, local), ...])
    t = 8.3
    for li, csz in enumerate(SYNC_RAMP):
        base_l = sum(SYNC_RAMP[:li]) - WCOLS
        locals_ = range(max(0, base_l) // CHUNK, (base_l + csz) // CHUNK)
        for s in range(NBAND):
            if li == 0 and s == 1:
                # parallel scalar-queue dispatch, lands ~8.2
                events.append((8.2, [(s, l) for l in locals_ if l < LCH[s]]))
                continue
            events.append((t, [(s, l) for l in locals_ if l < LCH[s]]))
            t += 0.8
    t = 9.7
    off = sum(SYNC_RAMP) - WCOLS
    for csz in GP_RAMP:
        lo, hi = off // CHUNK, (off + csz) // CHUNK
        for s in range(NBAND):
            events.append((t, [(s, l) for l in range(lo, hi) if l < LCH[s]]))
            t += 0.7
        off += csz
    events.sort(key=lambda e: e[0])
    return [sl for _, sls in events for sl in sls]

ARR = _arrival_order()
assert len(ARR) == NCHUNK and len(set(ARR)) == NCHUNK

# phase 2 rereads chunks (all resident by then), so its consumption
# order is free: use strict band rotation so consecutive matmuls
# overlap in different PE row-bands (ARR's same-band runs would make
# phase 2 PE-bound at ~1.7us/round vs the 1.12us copy cadence).
_INV = {sl: c for c, sl in enumerate(ARR)}
P2C = [_INV[(j % NBAND, j // NBAND)] for j in range(NBAND * max(LCH))
       if j // NBAND < LCH[j % NBAND]]
assert len(P2C) == NCHUNK

# copy-engine assignment per round: 0 -> DVE, 1 -> ACT. ACT is slightly
# faster per element, so it takes 25 of 48.
ENG_OF = [(1 if r % 2 else 0) for r in range(NROUND)]
for _r in range(NR1, NROUND):
    ENG_OF[_r] = 1 - ENG_OF[_r]

# per-engine staging: each engine's copy items get consecutive o8
# columns (engine-major), so an output DMA waits on ONE engine's copies
# only -- a late copy on one engine can no longer head-of-line-block
# the other engine's output stream on the sync queue. Rounds 0, 1 and
# the last round are split into 1-bank items on BOTH engines so the
# pipeline starts and drains faster; other rounds are one 2-bank item
# on ENG_OF[r]. Items pack greedily into staging tiles of <= 4 banks.
SPLIT_ROUNDS = frozenset([0, 1, NROUND - 1])
ITEMS = [[], []]
for _r in range(NROUND):
    if _r in SPLIT_ROUNDS:
        ITEMS[0].append((_r, 0, 1))
        ITEMS[1].append((_r, 1, 1))
    else:
        ITEMS[ENG_OF[_r]].append((_r, 0, 2))
ITEM_COL = {}
_col = 0
for _e in (0, 1):
    for _r, _b, _nb in ITEMS[_e]:
        ITEM_COL[(_r, _b)] = _col
        _col += _nb * CHUNK
assert _col == OCOLS
TILES = [[], []]
for _e in (0, 1):
    cur, nb = [], 0
    for _it in ITEMS[_e]:
        if nb + _it[2] > 4:
            TILES[_e].append(cur)
            cur, nb = [], 0
        cur.append(_it)
        nb += _it[2]
    if cur:
        TILES[_e].append(cur)
LAST_OF_TILE = {t[-1][:2]: t for _e in (0, 1) for t in TILES[_e]}

_NC_CACHE = {}


def _build_nc(out_bufs=8, eng_of=None):
    eng_of = list(eng_of) if eng_of is not None else list(ENG_OF)
    key = (out_bufs, tuple(eng_of))
    if key in _NC_CACHE:
        return _NC_CACHE[key]
    nc = bacc.Bacc(
        "TRN2",
        target_bir_lowering=False,
        debug=False,
        enable_asserts=False,
        num_devices=NCORES,
        enable_partition_id=False,
    )
    f16 = mybir.dt.float16
    f32 = mybir.dt.float32
    f8 = mybir.dt.float8e3
    q = nc.dram_tensor("q", [K, NBAND * SCOLS], f16, kind="ExternalInput").ap()
    o8 = nc.dram_tensor("o8", [128, OCOLS], f8, kind="ExternalOutput").ap()

    with tile.TileContext(nc) as tc:
        with (
            tc.tile_pool(name="qp", bufs=1) as qpool,
            tc.tile_pool(name="ps", bufs=1, space=bass.MemorySpace.PSUM) as pspool,
            tc.tile_pool(name="ot", bufs=out_bufs) as opool,
        ):
            qw = qpool.tile([BASES[-1] + K, SCOLS], f16)
            # strip s lives at partition base BASES[s]; dispatch size-major
            # (all strips' level k before level k+1)
            offs = [0] * NBAND
            for li, (ramp, eng) in enumerate(
                    ((SYNC_RAMP, nc.sync), (GP_RAMP, nc.gpsimd))):
                for ci, csz in enumerate(ramp):
                    for s, base in enumerate(BASES):
                        off = offs[s]
                        # strip 1's first chunk rides the idle scalar
                        # queue so the three round-0/1 loads parallelize
                        e = nc.scalar if (li == 0 and ci == 0 and s == 1) \
                            else eng
                        e.dma_start(
                            out=qw[base:base + K, off:off + csz],
                            in_=q[:, s * SCOLS + off:s * SCOLS + off + csz])
                        offs[s] += csz

            ps = pspool.tile([128, NBANK * CHUNK], f32)

            def mov(c):
                """moving AP for node-chunk c, slot ARR[c] = (strip, local)."""
                s, l = ARR[c]
                lo = WCOLS + l * CHUNK
                return qw[BASES[s]:BASES[s] + K, lo:lo + CHUNK]

            def w1(c):
                b = BASES[ARR[c][0]]
                return qw[b:b + K, 0:128]

            def w2(c):
                b = BASES[ARR[c][0]]
                return qw[b:b + K, 128:EMB]

            # per-engine staging tiles (<= 4 banks of items); a tile's
            # DMA waits on one engine's copies only
            etile = [None, None]
            eoff = [0, 0]
            for r in range(NROUND):
                poff = (r % 4) * GROUP
                if r < NR1:
                    for kk in range(2):
                        c = 2 * r + kk
                        nc.tensor.matmul(
                            ps[:, poff + kk * CHUNK:poff + (kk + 1) * CHUNK],
                            w1(c), mov(c), start=True, stop=True,
                        )
                else:
                    for kk in range(2):
                        i0 = 4 * (r - NR1) + 2 * kk
                        ca, cb = P2C[i0], P2C[i0 + 1]
                        dst = ps[:, poff + kk * CHUNK:poff + (kk + 1) * CHUNK]
                        nc.tensor.matmul(
                            dst[0:64, :], w2(ca), mov(ca),
                            start=True, stop=True,
                        )
                        nc.tensor.matmul(
                            dst[64:128, :], w2(cb), mov(cb),
                            start=True, stop=True,
                        )
                if r in SPLIT_ROUNDS:
                    ritems = [(0, (r, 0, 1)), (1, (r, 1, 1))]
                else:
                    ritems = [(eng_of[r], (r, 0, 2))]
                for e, it in ritems:
                    _, b, nb = it
                    if etile[e] is None:
                        cap = sum(x[2] for x in
                                  next(t for t in TILES[e] if t[0] == it))
                        etile[e] = opool.tile([128, cap * CHUNK], f8,
                                              name=f"ot{e}")
                        eoff[e] = 0
                    src = ps[:, poff + b * CHUNK:poff + (b + nb) * CHUNK]
                    dst = etile[e][:, eoff[e]:eoff[e] + nb * CHUNK]
                    if e:
                        nc.scalar.copy(dst, src)
                    else:
                        nc.vector.tensor_copy(dst, src)
                    eoff[e] += nb * CHUNK
                    tile_items = LAST_OF_TILE.get((r, b))
                    if tile_items is not None and \
                            sum(x[2] for x in tile_items) * CHUNK == eoff[e]:
                        col = ITEM_COL[tile_items[0][:2]]
                        nc.sync.dma_start(
                            out=o8[:, col:col + eoff[e]], in_=etile[e][:])
                        etile[e] = None
    nc.compile()
    _NC_CACHE[key] = nc
    return nc


def _host_prep(x, conv_w, gcn_w, gcn_b):
    x = np.asarray(x, dtype=np.float32)
    conv_w = np.asarray(conv_w, dtype=np.float32)
    gcn_w = np.asarray(gcn_w, dtype=np.float32)
    gcn_b = np.asarray(gcn_b, dtype=np.float32)

    # patches P[b, k, n]: k = (cin, ki, kj), n = r*WG + c
    P = np.ascontiguousarray(
        x.reshape(B, CIN, HG, 2, WG, 2).transpose(0, 1, 3, 5, 2, 4)
    ).reshape(B, 12, N)

    # degrees with self-loops; grid edges exist only for batch 0
    nbr = np.full((HG, WG), 4.0, np.float32)
    nbr[0, :] -= 1; nbr[-1, :] -= 1; nbr[:, 0] -= 1; nbr[:, -1] -= 1
    deg = nbr + 1.0
    deg[HG - 2, WG - 2] += 1.0          # the module's trailing extra edge
    dr = (1.0 / np.sqrt(deg)).ravel()    # dinv per node

    # batch-0 aggregation applied to the patch rows (commutes with the matmul)
    z = (dr[None, :] * P[0]).reshape(12, HG, WG)
    s = z.copy()                          # self-loop term
    s[:, 1:, :] += z[:, :-1, :]
    s[:, :-1, :] += z[:, 1:, :]
    s[:, :, 1:] += z[:, :, :-1]
    s[:, :, :-1] += z[:, :, 1:]
    s[:, HG - 2, WG - 2] += z[:, HG - 1, WG - 1]
    Q0 = dr[None, :] * s.reshape(12, N)

    Q = np.empty((K, BN), np.float32)
    Q[:12, :N] = Q0
    Q[:12, N:] = P[1:].transpose(1, 0, 2).reshape(12, 3 * N)
    Q[12, :] = 1.0                        # bias row

    Wcomb = (conv_w.reshape(EMB, 12).astype(np.float64).T
             @ gcn_w.astype(np.float64)).astype(np.float32)
    Wfull = np.concatenate([Wcomb, gcn_b[None, :]], axis=0)  # (13, 192)
    return Q, Wfull


def _decode_core(o8core, inv):
    """[128, OCOLS] fp8 -> [ROWS, EMB] fp32 (item-major column layout)."""
    o = o8core.astype(np.float32) * inv
    out = np.empty((ROWS, EMB), np.float32)
    for e in (0, 1):
        for r, b, nb in ITEMS[e]:
            col = ITEM_COL[(r, b)]
            seg = o[:, col:col + nb * CHUNK].reshape(128, nb, CHUNK)
            for j in range(nb):
                kk = b + j
                if r < NR1:
                    c = 2 * r + kk
                    out[c * CHUNK:(c + 1) * CHUNK, 0:128] = seg[:, j].T
                else:
                    i0 = 4 * (r - NR1) + 2 * kk
                    ca, cb = P2C[i0], P2C[i0 + 1]
                    out[ca * CHUNK:(ca + 1) * CHUNK, 128:EMB] = \
                        seg[0:64, j].T
                    out[cb * CHUNK:(cb + 1) * CHUNK, 128:EMB] = \
                        seg[64:128, j].T
    return out


def kernel(x, conv_w, gcn_w, gcn_b, _trace=False, _nc_kwargs=None):
    Q, Wfull = _host_prep(x, conv_w, gcn_w, gcn_b)
    nc = _build_nc(**(_nc_kwargs or {}))
    W16 = (Wfull * FP8_SCALE).astype(np.float16)
    Q16 = Q.astype(np.float16)
    in_maps = []
    for c in range(NCORES):
        qc = Q16[:, c * ROWS:(c + 1) * ROWS].reshape(K, NCHUNK, CHUNK)
        qd = np.zeros((K, NBAND, SCOLS), np.float16)
        qd[:, :, :WCOLS] = W16[:, None, :]
        for ch in range(NCHUNK):
            s, l = ARR[ch]
            qd[:, s, WCOLS + l * CHUNK:WCOLS + (l + 1) * CHUNK] = qc[:, ch]
        in_maps.append({"q": qd.reshape(K, NBAND * SCOLS)})
    res = run_bass_kernel_spmd(nc, in_maps, list(range(NCORES)), trace=_trace)
    inv = np.float32(1.0 / FP8_SCALE)
    out = np.empty((NCORES, ROWS, EMB), np.float32)
    for c in range(NCORES):
        out[c] = _decode_core(res.results[c]["o8"], inv)
    out = out.reshape(B, N, EMB)
    if _trace:
        return out, res
    return out
